# revision 5
# baseline (speedup 1.0000x reference)
"""Trainium2 Bass kernel for multi-head GNN message passing.

Problem: h_out = segment_softmax-style attention over 800k edges,
100k nodes, DIM=64, 4 heads x 16 dims (see the problem's reference).

Strategy (8 NeuronCores, no collectives):
  * Shard edges by destination-node range: core i owns dst nodes
    [i*12500, (i+1)*12500) and all edges pointing into them, so each
    core's segment-sums are purely local and the full output is the
    concatenation of the per-core outputs.
  * Within a core, sort nodes by in-degree (descending) and pack them
    into 98 tiles of 128 nodes. Each tile t gets C_t = max in-degree
    chunks of 128 edge "slots": slot (t, j, p) holds the j-th incoming
    edge of the tile's p-th node (empty slots are zero-padded). Because
    slot partition == destination node, the per-edge -> per-node
    segment sum is a plain PSUM accumulation through an identity-matmul
    (no one-hot matrices, no gathers, no scatter).
  * The host uploads, per slot, the 64 edge_attr features and the 64
    src-node features interleaved on 128 partitions (bf16). The device
    then computes E/K/V per slot with three matmuls against
    block-diagonal weights, the attention score with DVE elementwise ops
    + ACT exp, messages V*score, and accumulates wV and Z in PSUM.
    Empty slots contribute exactly 1.0 to Z (exp(0)); the host-provided
    per-node pad count is subtracted from Z before the division.

All "device" work (projections, scores, exp, messages, segment sums,
normalization) runs on the NeuronCores; the host only reorders/pads/
casts input data and concatenates the output.
"""

import numpy as np
import ml_dtypes

import concourse.bass as bass
import concourse.tile as tile
from concourse import bacc, mybir
from concourse.bass_utils import run_bass_kernel_spmd

BF16 = ml_dtypes.bfloat16

N_NODES = 100_000
N_EDGES = 800_000
DIM = 64
H = 4
DH = 16
NCORES = 8
NPC = N_NODES // NCORES           # 12500 nodes per core
NT = (NPC + 127) // 128           # 98 node tiles per core
NPAD = NT * 128                   # 12544 rank slots per core
CSUB = 8                          # chunks per sub-tile (PSUM bank limit)
CLIP = 5.0 * np.sqrt(DH)          # clip(x/sqrt(Dh), -5, 5) == clip(x, -20, 20)/4


# ----------------------------------------------------------------------------
# host-side sharding / packing
# ----------------------------------------------------------------------------

def _shard(x, edge_attr, edge_index):
    """Compute the per-core slot layout and packed input arrays."""
    src = np.asarray(edge_index[0]).astype(np.int64)
    dst = np.asarray(edge_index[1]).astype(np.int64)
    x = np.ascontiguousarray(np.asarray(x, dtype=np.float32))
    ea = np.ascontiguousarray(np.asarray(edge_attr, dtype=np.float32))
    x_b = x.astype(BF16)
    ea_b = ea.astype(BF16)

    core_of_edge = dst // NPC
    per_core = []
    deg_rank_all = []
    for c in range(NCORES):
        eids = np.nonzero(core_of_edge == c)[0]
        dl = dst[eids] - c * NPC                      # local dst in [0, NPC)
        deg = np.bincount(dl, minlength=NPC)
        order = np.argsort(-deg, kind="stable")       # node of rank r
        rank_of = np.empty(NPC, np.int64)
        rank_of[order] = np.arange(NPC)
        rk = rank_of[dl]                              # rank of each edge's dst
        # j-index: position of the edge within its node's list
        sort_idx = np.argsort(rk, kind="stable")
        rk_sorted = rk[sort_idx]
        # cumcount within equal-rank runs
        first = np.r_[True, rk_sorted[1:] != rk_sorted[:-1]]
        idx = np.arange(len(rk_sorted))
        start = np.maximum.accumulate(np.where(first, idx, 0))
        j_sorted = idx - start
        deg_sorted = deg[order]                       # degree by rank
        per_core.append((eids[sort_idx], rk_sorted, j_sorted, order))
        deg_rank_all.append(deg_sorted)

    # SPMD-uniform chunk counts per tile (max over cores)
    Ct = np.zeros(NT, np.int64)
    for c in range(NCORES):
        d = deg_rank_all[c]
        d = np.r_[d, np.zeros(NPAD - NPC, np.int64)]
        Ct = np.maximum(Ct, d.reshape(NT, 128).max(axis=1))
    Ct = Ct.astype(int)
    off = np.r_[0, np.cumsum(Ct)]                     # chunk offset per tile
    S = int(off[-1]) * 128                            # slots per core

    in_maps = []
    unshard = []
    for c in range(NCORES):
        eids, rk, jj, order = per_core[c]
        t = rk // 128
        p = rk % 128
        cols = (off[t] + jj) * 128 + p                # slot column
        stat = np.zeros((128, S), BF16)
        stat[0:64, cols] = ea_b[eids].T
        stat[64:128, cols] = x_b[src[eids]].T

        d = deg_rank_all[c]
        xtq = np.zeros((64, NPAD), BF16)
        xtq[:, :NPC] = x_b[c * NPC + order].T

        zsub = np.zeros((128, NT), np.float32)
        degr = np.r_[d, np.zeros(NPAD - NPC, np.int64)].reshape(NT, 128)
        zsub[:, :] = (Ct[None, :] - degr.T).astype(np.float32)

        in_maps.append({"stat": stat, "xtq": xtq, "zsub": zsub})
        unshard.append(order)

    return in_maps, unshard, Ct, S


def _consts(Wq, Wk, We, Wv):
    Wq = np.asarray(Wq, np.float32); Wk = np.asarray(Wk, np.float32)
    We = np.asarray(We, np.float32); Wv = np.asarray(Wv, np.float32)
    wblk = np.zeros((128, 192), BF16)
    wblk[0:64, 0:64] = We.T.astype(BF16)
    wblk[64:128, 64:128] = Wk.T.astype(BF16)
    wblk[64:128, 128:192] = Wv.T.astype(BF16)
    wqt = Wq.T.astype(BF16)
    ident = np.eye(128, dtype=BF16)
    return {"wblk": wblk, "wqt": wqt, "ident": ident}


# ----------------------------------------------------------------------------
# device program
# ----------------------------------------------------------------------------

def _build_program(Ct, S):
    nc = bacc.Bacc("TRN2", target_bir_lowering=False, debug=False,
                   num_devices=NCORES)
    f32 = mybir.dt.float32
    bf16 = mybir.dt.bfloat16

    stat_d = nc.dram_tensor("stat", [128, S], bf16, kind="ExternalInput")
    xtq_d = nc.dram_tensor("xtq", [64, NPAD], bf16, kind="ExternalInput")
    zsub_d = nc.dram_tensor("zsub", [128, NT], f32, kind="ExternalInput")
    wblk_d = nc.dram_tensor("wblk", [128, 192], bf16, kind="ExternalInput")
    wqt_d = nc.dram_tensor("wqt", [64, 64], bf16, kind="ExternalInput")
    ident_d = nc.dram_tensor("ident", [128, 128], bf16, kind="ExternalInput")
    out_d = nc.dram_tensor("out", [128, NT * 64], f32, kind="ExternalOutput")

    Exp = mybir.ActivationFunctionType.Exp
    mult = mybir.AluOpType.mult
    addop = mybir.AluOpType.add

    with tile.TileContext(nc) as tc:
        with (
            tc.tile_pool(name="const", bufs=1) as constp,
            tc.tile_pool(name="pers", bufs=1) as persp,
            tc.tile_pool(name="stream", bufs=3) as streamp,
            tc.tile_pool(name="work", bufs=2) as workp,
            tc.tile_pool(name="psum", bufs=2, space="PSUM") as psump,
        ):
            wblk = constp.tile([128, 192], bf16, tag="wblk")
            nc.sync.dma_start(wblk[:], wblk_d[:])
            wqt = constp.tile([64, 64], bf16, tag="wqt")
            nc.sync.dma_start(wqt[:], wqt_d[:])
            ident = constp.tile([128, 128], bf16, tag="ident")
            nc.sync.dma_start(ident[:], ident_d[:])
            zsub = constp.tile([128, NT], f32, tag="zsub")
            nc.sync.dma_start(zsub[:], zsub_d[:])

            xtq = persp.tile([64, NPAD], bf16, tag="xtq")
            nc.sync.dma_start(xtq[:], xtq_d[:])
            q_sb = persp.tile([128, NT * 64], bf16, tag="q")
            acc_sb = persp.tile([128, NT * 68], f32, tag="acc")

            # ---- phase Q: Q[rank, :] = x[rank] @ Wq.T, node-rank-major ----
            for t in range(NT):
                qp = psump.tile([128, 68], f32, tag="acc_ps")
                nc.tensor.matmul(out=qp[:, 0:64],
                                 lhsT=xtq[:, t * 128:(t + 1) * 128],
                                 rhs=wqt[:], start=True, stop=True)
                nc.scalar.copy(q_sb[:, t * 64:(t + 1) * 64], qp[:, 0:64])

            # ---- main loop over node tiles ----
            off = np.r_[0, np.cumsum(Ct)].astype(int)
            for t in range(NT):
                C = int(Ct[t])
                av = acc_sb[:, t * 68:(t + 1) * 68]
                if C == 0:
                    nc.vector.memset(av, 0.0)
                    continue
                accp = psump.tile([128, 68], f32, tag="acc_ps")
                qt = q_sb[:, t * 64:(t + 1) * 64]
                nsub = (C + CSUB - 1) // CSUB
                for s in range(nsub):
                    c0 = s * CSUB
                    cs = min(CSUB, C - c0)
                    col0 = (off[t] + c0) * 128
                    st = streamp.tile([128, CSUB * 128], bf16, tag="stat")
                    nc.sync.dma_start(st[:, 0:cs * 128],
                                      stat_d[:, col0:col0 + cs * 128])
                    ep = psump.tile([128, CSUB * 64], f32, tag="E")
                    kp = psump.tile([128, CSUB * 64], f32, tag="K")
                    vp = psump.tile([128, CSUB * 64], f32, tag="V")
                    for j in range(cs):
                        lh = st[:, j * 128:(j + 1) * 128]
                        nc.tensor.matmul(out=ep[:, j * 64:(j + 1) * 64],
                                         lhsT=lh, rhs=wblk[:, 0:64],
                                         start=True, stop=True)
                        nc.tensor.matmul(out=kp[:, j * 64:(j + 1) * 64],
                                         lhsT=lh, rhs=wblk[:, 64:128],
                                         start=True, stop=True)
                        nc.tensor.matmul(out=vp[:, j * 64:(j + 1) * 64],
                                         lhsT=lh, rhs=wblk[:, 128:192],
                                         start=True, stop=True)
                    fd = cs * 64
                    p2 = workp.tile([128, CSUB * 64], f32, tag="p2")
                    nc.vector.tensor_tensor(
                        out=p2[:, 0:fd].rearrange("p (c d) -> p c d", d=64),
                        in0=kp[:, 0:fd].rearrange("p (c d) -> p c d", d=64),
                        in1=qt.unsqueeze(1).to_broadcast([128, cs, 64]),
                        op=mult)
                    p3 = workp.tile([128, CSUB * 64], f32, tag="p3")
                    nc.vector.tensor_tensor(out=p3[:, 0:fd], in0=p2[:, 0:fd],
                                            in1=ep[:, 0:fd], op=mult)
                    sc = workp.tile([128, CSUB * 4], f32, tag="sc")
                    nc.vector.tensor_reduce(
                        out=sc[:, 0:cs * 4].rearrange("p (c h) -> p c h", h=4),
                        in_=p3[:, 0:fd].rearrange("p (c h d) -> p c h d",
                                                  h=4, d=16),
                        axis=mybir.AxisListType.X, op=addop)
                    scc = workp.tile([128, CSUB * 4], f32, tag="scc")
                    nc.vector.tensor_scalar(
                        out=scc[:, 0:cs * 4], in0=sc[:, 0:cs * 4],
                        scalar1=-CLIP, scalar2=CLIP,
                        op0=mybir.AluOpType.max, op1=mybir.AluOpType.min)
                    mb = workp.tile([128, CSUB, 68], bf16, tag="mb")
                    nc.scalar.activation(
                        out=mb[:, 0:cs, 64:68],
                        in_=scc[:, 0:cs * 4].rearrange("p (c h) -> p c h", h=4),
                        func=Exp, scale=float(1.0 / np.sqrt(DH)))
                    nc.vector.tensor_tensor(
                        out=mb[:, 0:cs, 0:64].rearrange("p c (h d) -> p c h d",
                                                        h=4),
                        in0=vp[:, 0:fd].rearrange("p (c h d) -> p c h d",
                                                  h=4, d=16),
                        in1=mb[:, 0:cs, 64:68].unsqueeze(3)
                            .to_broadcast([128, cs, 4, 16]),
                        op=mult)
                    for j in range(cs):
                        nc.tensor.matmul(out=accp[:],
                                         lhsT=ident[:], rhs=mb[:, j, :],
                                         start=(c0 + j == 0),
                                         stop=(c0 + j == C - 1))
                nc.vector.tensor_copy(av, accp[:])

            # ---- normalize: out = wV / (Z - zsub + 1e-6) ----
            accv = acc_sb[:].rearrange("p (t w) -> p t w", w=68)
            zc = persp.tile([128, NT * 4], f32, tag="zc")
            zcv = zc[:].rearrange("p (t h) -> p t h", h=4)
            nc.vector.tensor_tensor(
                out=zcv, in0=accv[:, :, 64:68],
                in1=zsub[:].unsqueeze(2).to_broadcast([128, NT, 4]),
                op=mybir.AluOpType.subtract)
            nc.vector.tensor_scalar_add(zc[:], zc[:], 1e-6)
            rz = persp.tile([128, NT * 4], f32, tag="rz")
            nc.vector.reciprocal(out=rz[:], in_=zc[:])
            nc.vector.tensor_tensor(
                out=accv[:, :, 0:64].rearrange("p t (h d) -> p t h d", h=4),
                in0=accv[:, :, 0:64].rearrange("p t (h d) -> p t h d", h=4),
                in1=rz[:].rearrange("p (t h) -> p t h", h=4).unsqueeze(3)
                      .to_broadcast([128, NT, 4, 16]),
                op=mult)
            nc.sync.dma_start(
                out_d.ap().rearrange("p (t d) -> p t d", d=64),
                accv[:, :, 0:64])

    nc.compile()
    return nc


# ----------------------------------------------------------------------------
# public entry point
# ----------------------------------------------------------------------------

_CACHE = {}


def prepare(x, edge_attr, Wq, Wk, We, Wv, edge_index):
    """Host prep + (cached) program build. Returns (nc, in_maps, unshard)."""
    in_maps, unshard, Ct, S = _shard(x, edge_attr, edge_index)
    consts = _consts(Wq, Wk, We, Wv)
    for m in in_maps:
        m.update(consts)
    key = (tuple(Ct), S)
    if key not in _CACHE:
        _CACHE[key] = _build_program(Ct, S)
    return _CACHE[key], in_maps, unshard


def unshard_output(results, unshard):
    h = np.empty((N_NODES, DIM), np.float32)
    for c in range(NCORES):
        o = results[c]["out"].reshape(128, NT, 64)
        r = np.arange(NPC)
        rows = o[r % 128, r // 128]          # [NPC, 64] by rank
        h[c * NPC + unshard[c]] = rows
    return h


def kernel(x, edge_attr, Wq, Wk, We, Wv, edge_index):
    nc, in_maps, unshard = prepare(x, edge_attr, Wq, Wk, We, Wv, edge_index)
    res = run_bass_kernel_spmd(nc, in_maps, core_ids=list(range(NCORES)))
    return unshard_output(res.results, unshard)


# revision 10
# speedup vs baseline: 188.0099x; 188.0099x over previous
"""Trainium2 Bass kernel for multi-head GNN message passing.

Problem: h_out = segment_softmax-style attention over 800k edges,
100k nodes, DIM=64, 4 heads x 16 dims (see the problem's reference).

Strategy (8 NeuronCores, no collectives):
  * Shard edges by destination-node range: core i owns dst nodes
    [i*12500, (i+1)*12500) and all edges pointing into them, so each
    core's segment-sums are purely local and the full output is the
    concatenation of the per-core outputs.
  * Within a core, sort nodes by in-degree (descending) and pack them
    into 98 tiles of 128 nodes. Each tile t gets C_t = max in-degree
    chunks of 128 edge "slots": slot (t, j, p) holds the j-th incoming
    edge of the tile's p-th node (empty slots are zero-padded). Because
    slot partition == destination node, the per-edge -> per-node
    segment sum is a plain PSUM accumulation through an identity-matmul
    (no one-hot matrices, no gathers, no scatter).
  * The host uploads, per slot, the 64 edge_attr features and the 64
    src-node features interleaved on 128 partitions (bf16). The device
    then computes E/K/V per slot with three matmuls against
    block-diagonal weights, the attention score with DVE elementwise ops
    + ACT exp, messages V*score, and accumulates wV and Z in PSUM.
    Empty slots contribute exactly 1.0 to Z (exp(0)); the host-provided
    per-node pad count is subtracted from Z before the division.

All "device" work (projections, scores, exp, messages, segment sums,
normalization) runs on the NeuronCores; the host only reorders/pads/
casts input data and concatenates the output.
"""

import numpy as np
import ml_dtypes

import concourse.bass as bass
import concourse.tile as tile
from concourse import bacc, mybir
from concourse.bass_utils import run_bass_kernel_spmd

BF16 = ml_dtypes.bfloat16

N_NODES = 100_000
N_EDGES = 800_000
DIM = 64
H = 4
DH = 16
NCORES = 8
NPC = N_NODES // NCORES           # 12500 nodes per core
NT = (NPC + 127) // 128           # 98 node tiles per core
NPAD = NT * 128                   # 12544 rank slots per core
CSUB = 8                          # chunks per sub-tile (PSUM bank limit)
CLIP = 5.0 * np.sqrt(DH)          # clip(x/sqrt(Dh), -5, 5) == clip(x, -20, 20)/4


# ----------------------------------------------------------------------------
# host-side sharding / packing
# ----------------------------------------------------------------------------

def _shard(x, edge_attr, edge_index):
    """Compute the per-core slot layout and packed input arrays."""
    src = np.asarray(edge_index[0]).astype(np.int64)
    dst = np.asarray(edge_index[1]).astype(np.int64)
    x = np.ascontiguousarray(np.asarray(x, dtype=np.float32))
    ea = np.ascontiguousarray(np.asarray(edge_attr, dtype=np.float32))
    x_b = x.astype(BF16)
    ea_b = ea.astype(BF16)

    core_of_edge = dst // NPC
    per_core = []
    deg_rank_all = []
    for c in range(NCORES):
        eids = np.nonzero(core_of_edge == c)[0]
        dl = dst[eids] - c * NPC                      # local dst in [0, NPC)
        deg = np.bincount(dl, minlength=NPC)
        order = np.argsort(-deg, kind="stable")       # node of rank r
        rank_of = np.empty(NPC, np.int64)
        rank_of[order] = np.arange(NPC)
        rk = rank_of[dl]                              # rank of each edge's dst
        # j-index: position of the edge within its node's list
        sort_idx = np.argsort(rk, kind="stable")
        rk_sorted = rk[sort_idx]
        # cumcount within equal-rank runs
        first = np.r_[True, rk_sorted[1:] != rk_sorted[:-1]]
        idx = np.arange(len(rk_sorted))
        start = np.maximum.accumulate(np.where(first, idx, 0))
        j_sorted = idx - start
        deg_sorted = deg[order]                       # degree by rank
        per_core.append((eids[sort_idx], rk_sorted, j_sorted, order))
        deg_rank_all.append(deg_sorted)

    # SPMD-uniform chunk counts per tile (max over cores)
    Ct = np.zeros(NT, np.int64)
    for c in range(NCORES):
        d = deg_rank_all[c]
        d = np.r_[d, np.zeros(NPAD - NPC, np.int64)]
        Ct = np.maximum(Ct, d.reshape(NT, 128).max(axis=1))
    Ct = Ct.astype(int)
    off = np.r_[0, np.cumsum(Ct)]                     # chunk offset per tile
    S = int(off[-1]) * 128                            # slots per core

    in_maps = []
    unshard = []
    for c in range(NCORES):
        eids, rk, jj, order = per_core[c]
        t = rk // 128
        p = rk % 128
        cols = (off[t] + jj) * 128 + p                # slot column
        stat = np.zeros((128, S), BF16)
        stat[0:64, cols] = ea_b[eids].T
        stat[64:128, cols] = x_b[src[eids]].T

        d = deg_rank_all[c]
        xtq = np.zeros((64, NPAD), BF16)
        xtq[:, :NPC] = x_b[c * NPC + order].T

        zsub = np.zeros((128, NT), np.float32)
        degr = np.r_[d, np.zeros(NPAD - NPC, np.int64)].reshape(NT, 128)
        zsub[:, :] = (Ct[None, :] - degr.T).astype(np.float32)

        in_maps.append({"stat": stat, "xtq": xtq, "zsub": zsub})
        unshard.append(order)

    return in_maps, unshard, Ct, S


def _consts(Wq, Wk, We, Wv):
    Wq = np.asarray(Wq, np.float32); Wk = np.asarray(Wk, np.float32)
    We = np.asarray(We, np.float32); Wv = np.asarray(Wv, np.float32)
    wblk = np.zeros((128, 192), BF16)
    wblk[0:64, 0:64] = We.T.astype(BF16)
    wblk[64:128, 64:128] = Wk.T.astype(BF16)
    wblk[64:128, 128:192] = Wv.T.astype(BF16)
    wqt = Wq.T.astype(BF16)
    ident = np.eye(128, dtype=BF16)
    return {"wblk": wblk, "wqt": wqt, "ident": ident}


# ----------------------------------------------------------------------------
# device program
# ----------------------------------------------------------------------------

def _build_program(Ct, S, reps=1):
    nc = bacc.Bacc("TRN2", target_bir_lowering=False, debug=False,
                   num_devices=NCORES)
    f32 = mybir.dt.float32
    bf16 = mybir.dt.bfloat16

    stat_d = nc.dram_tensor("stat", [128, S], bf16, kind="ExternalInput")
    xtq_d = nc.dram_tensor("xtq", [64, NPAD], bf16, kind="ExternalInput")
    zsub_d = nc.dram_tensor("zsub", [128, NT], f32, kind="ExternalInput")
    wblk_d = nc.dram_tensor("wblk", [128, 192], bf16, kind="ExternalInput")
    wqt_d = nc.dram_tensor("wqt", [64, 64], bf16, kind="ExternalInput")
    ident_d = nc.dram_tensor("ident", [128, 128], bf16, kind="ExternalInput")
    out_d = nc.dram_tensor("out", [128, NT * 64], f32, kind="ExternalOutput")

    Exp = mybir.ActivationFunctionType.Exp
    mult = mybir.AluOpType.mult
    addop = mybir.AluOpType.add
    off = np.r_[0, np.cumsum(Ct)].astype(int)

    def emit_pass(nc, constp, persp, streamp, workp, psump,
                  wblk, wqt, ident, zsub, xtq, q_sb, acc_sb):
        # ---- phase Q: Q[rank, :] = x[rank] @ Wq.T, node-rank-major ----
        for t in range(NT):
            qp = psump.tile([128, 68], f32, tag="acc_ps")
            nc.tensor.matmul(out=qp[:, 0:64],
                             lhsT=xtq[:, t * 128:(t + 1) * 128],
                             rhs=wqt[:], start=True, stop=True)
            nc.scalar.copy(q_sb[:, t * 64:(t + 1) * 64], qp[:, 0:64])

        # ---- main loop over node tiles ----
        for t in range(NT):
            C = int(Ct[t])
            av = acc_sb[:, t * 68:(t + 1) * 68]
            if C == 0:
                nc.vector.memset(av, 0.0)
                continue
            accp = psump.tile([128, 68], f32, tag="acc_ps")
            qt = q_sb[:, t * 64:(t + 1) * 64]
            nsub = (C + CSUB - 1) // CSUB
            for s in range(nsub):
                c0 = s * CSUB
                cs = min(CSUB, C - c0)
                col0 = (off[t] + c0) * 128
                st = streamp.tile([128, CSUB * 128], bf16, tag="stat")
                nc.sync.dma_start(st[:, 0:cs * 128],
                                  stat_d[:, col0:col0 + cs * 128])
                ep = psump.tile([128, CSUB * 64], f32, tag="E")
                kp = psump.tile([128, CSUB * 64], f32, tag="K")
                vp = psump.tile([128, CSUB * 64], f32, tag="V")
                for j in range(cs):
                    lh = st[:, j * 128:(j + 1) * 128]
                    nc.tensor.matmul(out=ep[:, j * 64:(j + 1) * 64],
                                     lhsT=lh, rhs=wblk[:, 0:64],
                                     start=True, stop=True)
                    nc.tensor.matmul(out=kp[:, j * 64:(j + 1) * 64],
                                     lhsT=lh, rhs=wblk[:, 64:128],
                                     start=True, stop=True)
                    nc.tensor.matmul(out=vp[:, j * 64:(j + 1) * 64],
                                     lhsT=lh, rhs=wblk[:, 128:192],
                                     start=True, stop=True)
                fd = cs * 64
                p2 = workp.tile([128, CSUB * 64], f32, tag="p2")
                nc.vector.tensor_tensor(
                    out=p2[:, 0:fd].rearrange("p (c d) -> p c d", d=64),
                    in0=kp[:, 0:fd].rearrange("p (c d) -> p c d", d=64),
                    in1=qt.unsqueeze(1).to_broadcast([128, cs, 64]),
                    op=mult)
                p3 = workp.tile([128, CSUB * 64], f32, tag="p3")
                nc.vector.tensor_tensor(out=p3[:, 0:fd], in0=p2[:, 0:fd],
                                        in1=ep[:, 0:fd], op=mult)
                sc = workp.tile([128, CSUB * 4], f32, tag="sc")
                nc.vector.tensor_reduce(
                    out=sc[:, 0:cs * 4].rearrange("p (c h) -> p c h", h=4),
                    in_=p3[:, 0:fd].rearrange("p (c h d) -> p c h d",
                                              h=4, d=16),
                    axis=mybir.AxisListType.X, op=addop)
                scc = workp.tile([128, CSUB * 4], f32, tag="scc")
                nc.vector.tensor_scalar(
                    out=scc[:, 0:cs * 4], in0=sc[:, 0:cs * 4],
                    scalar1=-CLIP, scalar2=CLIP,
                    op0=mybir.AluOpType.max, op1=mybir.AluOpType.min)
                mb = workp.tile([128, CSUB, 68], bf16, tag="mb")
                nc.scalar.activation(
                    out=mb[:, 0:cs, 64:68],
                    in_=scc[:, 0:cs * 4].rearrange("p (c h) -> p c h", h=4),
                    func=Exp, scale=float(1.0 / np.sqrt(DH)))
                nc.vector.tensor_tensor(
                    out=mb[:, 0:cs, 0:64].rearrange("p c (h d) -> p c h d",
                                                    h=4),
                    in0=vp[:, 0:fd].rearrange("p (c h d) -> p c h d",
                                              h=4, d=16),
                    in1=mb[:, 0:cs, 64:68].unsqueeze(3)
                        .to_broadcast([128, cs, 4, 16]),
                    op=mult)
                for j in range(cs):
                    nc.tensor.matmul(out=accp[:],
                                     lhsT=ident[:], rhs=mb[:, j, :],
                                     start=(c0 + j == 0),
                                     stop=(c0 + j == C - 1))
            nc.vector.tensor_copy(av, accp[:])

        # ---- normalize: out = wV / (Z - zsub + 1e-6) ----
        accv = acc_sb[:].rearrange("p (t w) -> p t w", w=68)
        zc = persp.tile([128, NT * 4], f32, tag="zc")
        zcv = zc[:].rearrange("p (t h) -> p t h", h=4)
        nc.vector.tensor_tensor(
            out=zcv, in0=accv[:, :, 64:68],
            in1=zsub[:].unsqueeze(2).to_broadcast([128, NT, 4]),
            op=mybir.AluOpType.subtract)
        nc.vector.tensor_scalar_add(zc[:], zc[:], 1e-6)
        rz = persp.tile([128, NT * 4], f32, tag="rz")
        nc.vector.reciprocal(out=rz[:], in_=zc[:])
        nc.vector.tensor_tensor(
            out=accv[:, :, 0:64].rearrange("p t (h d) -> p t h d", h=4),
            in0=accv[:, :, 0:64].rearrange("p t (h d) -> p t h d", h=4),
            in1=rz[:].rearrange("p (t h) -> p t h", h=4).unsqueeze(3)
                  .to_broadcast([128, NT, 4, 16]),
            op=mult)
        nc.sync.dma_start(
            out_d.ap().rearrange("p (t d) -> p t d", d=64),
            accv[:, :, 0:64])

    with tile.TileContext(nc) as tc:
        with (
            tc.tile_pool(name="const", bufs=1) as constp,
            tc.tile_pool(name="pers", bufs=1) as persp,
            tc.tile_pool(name="stream", bufs=3) as streamp,
            tc.tile_pool(name="work", bufs=2) as workp,
            tc.tile_pool(name="psum", bufs=2, space="PSUM") as psump,
        ):
            wblk = constp.tile([128, 192], bf16, tag="wblk")
            nc.sync.dma_start(wblk[:], wblk_d[:])
            wqt = constp.tile([64, 64], bf16, tag="wqt")
            nc.sync.dma_start(wqt[:], wqt_d[:])
            ident = constp.tile([128, 128], bf16, tag="ident")
            nc.sync.dma_start(ident[:], ident_d[:])
            zsub = constp.tile([128, NT], f32, tag="zsub")
            nc.sync.dma_start(zsub[:], zsub_d[:])

            xtq = persp.tile([64, NPAD], bf16, tag="xtq")
            nc.sync.dma_start(xtq[:], xtq_d[:])
            q_sb = persp.tile([128, NT * 64], bf16, tag="q")
            acc_sb = persp.tile([128, NT * 68], f32, tag="acc")

            for _rep in range(reps):
                emit_pass(nc, constp, persp, streamp, workp, psump,
                          wblk, wqt, ident, zsub, xtq, q_sb, acc_sb)

    nc.compile()
    return nc


# ----------------------------------------------------------------------------
# public entry point
# ----------------------------------------------------------------------------

_CACHE = {}


def prepare(x, edge_attr, Wq, Wk, We, Wv, edge_index, reps=1):
    """Host prep + (cached) program build.

    Returns (nc, in_maps, unshard, Ct, S)."""
    in_maps, unshard, Ct, S = _shard(x, edge_attr, edge_index)
    consts = _consts(Wq, Wk, We, Wv)
    for m in in_maps:
        m.update(consts)
    key = (tuple(Ct), S, reps)
    if key not in _CACHE:
        _CACHE[key] = _build_program(Ct, S, reps=reps)
    return _CACHE[key], in_maps, unshard, Ct, S


def unshard_output(results, unshard):
    h = np.empty((N_NODES, DIM), np.float32)
    for c in range(NCORES):
        o = results[c]["out"].reshape(128, NT, 64)
        r = np.arange(NPC)
        rows = o[r % 128, r // 128]          # [NPC, 64] by rank
        h[c * NPC + unshard[c]] = rows
    return h


def kernel(x, edge_attr, Wq, Wk, We, Wv, edge_index):
    nc, in_maps, unshard, _, _ = prepare(x, edge_attr, Wq, Wk, We, Wv,
                                         edge_index)
    res = run_bass_kernel_spmd(nc, in_maps, core_ids=list(range(NCORES)))
    return unshard_output(res.results, unshard)


# revision 15
# speedup vs baseline: 440.5175x; 2.3431x over previous
"""Trainium2 Bass kernel for multi-head GNN message passing.

Problem: h_out = segment_softmax-style attention over 800k edges,
100k nodes, DIM=64, 4 heads x 16 dims (see the problem's reference).

Strategy (8 NeuronCores, no collectives):
  * Shard edges by destination-node range: core i owns dst nodes
    [i*12500, (i+1)*12500) and all edges pointing into them, so each
    core's segment-sums are purely local and the full output is the
    concatenation of the per-core outputs.
  * Within a core, sort nodes by in-degree (descending) and pack them
    into 98 tiles of 128 nodes. Each tile t gets C_t = max in-degree
    chunks of 128 edge "slots": slot (t, j, p) holds the j-th incoming
    edge of the tile's p-th node (empty slots are zero-padded). Because
    slot partition == destination node, the per-edge -> per-node
    segment sum is a plain PSUM accumulation through an identity-matmul
    (no one-hot matrices, no gathers, no scatter).
  * The host uploads, per slot, the 64 edge_attr features and the 64
    src-node features interleaved on 128 partitions (bf16). The device
    then computes E/K/V per slot with three matmuls against
    block-diagonal weights, the attention score with DVE elementwise ops
    + ACT exp, messages V*score, and accumulates wV and Z in PSUM.
    Empty slots contribute exactly 1.0 to Z (exp(0)); the host-provided
    per-node pad count is subtracted from Z before the division.

All "device" work (projections, scores, exp, messages, segment sums,
normalization) runs on the NeuronCores; the host only reorders/pads/
casts input data and concatenates the output.
"""

import numpy as np
import ml_dtypes

import concourse.bass as bass
import concourse.tile as tile
from concourse import bacc, mybir
from concourse.bass_utils import run_bass_kernel_spmd

F16 = np.float16

N_NODES = 100_000
N_EDGES = 800_000
DIM = 64
H = 4
DH = 16
NCORES = 8
NPC = N_NODES // NCORES           # 12500 nodes per core
NT = (NPC + 127) // 128           # 98 node tiles per core
NPAD = NT * 128                   # 12544 rank slots per core
CSUB = 8                          # chunks per sub-tile (PSUM bank limit)
P3_ON_POOL = True
TREE_REDUCE = True
CLIP = 5.0 * np.sqrt(DH)          # clip(x/sqrt(Dh), -5, 5) == clip(x, -20, 20)/4


# ----------------------------------------------------------------------------
# host-side sharding / packing
# ----------------------------------------------------------------------------

def _shard(x, edge_attr, edge_index):
    """Compute the per-core slot layout and packed input arrays."""
    src = np.asarray(edge_index[0]).astype(np.int64)
    dst = np.asarray(edge_index[1]).astype(np.int64)
    x = np.ascontiguousarray(np.asarray(x, dtype=np.float32))
    ea = np.ascontiguousarray(np.asarray(edge_attr, dtype=np.float32))
    x_b = x.astype(F16)
    ea_b = ea.astype(F16)

    core_of_edge = dst // NPC
    per_core = []
    deg_rank_all = []
    for c in range(NCORES):
        eids = np.nonzero(core_of_edge == c)[0]
        dl = dst[eids] - c * NPC                      # local dst in [0, NPC)
        deg = np.bincount(dl, minlength=NPC)
        order = np.argsort(-deg, kind="stable")       # node of rank r
        rank_of = np.empty(NPC, np.int64)
        rank_of[order] = np.arange(NPC)
        rk = rank_of[dl]                              # rank of each edge's dst
        # j-index: position of the edge within its node's list
        sort_idx = np.argsort(rk, kind="stable")
        rk_sorted = rk[sort_idx]
        # cumcount within equal-rank runs
        first = np.r_[True, rk_sorted[1:] != rk_sorted[:-1]]
        idx = np.arange(len(rk_sorted))
        start = np.maximum.accumulate(np.where(first, idx, 0))
        j_sorted = idx - start
        deg_sorted = deg[order]                       # degree by rank
        per_core.append((eids[sort_idx], rk_sorted, j_sorted, order))
        deg_rank_all.append(deg_sorted)

    # SPMD-uniform chunk counts per tile (max over cores)
    Ct = np.zeros(NT, np.int64)
    for c in range(NCORES):
        d = deg_rank_all[c]
        d = np.r_[d, np.zeros(NPAD - NPC, np.int64)]
        Ct = np.maximum(Ct, d.reshape(NT, 128).max(axis=1))
    Ct = Ct.astype(int)
    off = np.r_[0, np.cumsum(Ct)]                     # chunk offset per tile
    S = int(off[-1]) * 128                            # slots per core

    in_maps = []
    unshard = []
    for c in range(NCORES):
        eids, rk, jj, order = per_core[c]
        t = rk // 128
        p = rk % 128
        cols = (off[t] + jj) * 128 + p                # slot column
        stat = np.zeros((128, S), F16)
        stat[0:64, cols] = ea_b[eids].T
        stat[64:128, cols] = x_b[src[eids]].T

        d = deg_rank_all[c]
        xtq = np.zeros((64, NPAD), F16)
        xtq[:, :NPC] = x_b[c * NPC + order].T

        zsub = np.zeros((128, NT), np.float32)
        degr = np.r_[d, np.zeros(NPAD - NPC, np.int64)].reshape(NT, 128)
        zsub[:, :] = (Ct[None, :] - degr.T).astype(np.float32)

        in_maps.append({"stat": stat, "xtq": xtq, "zsub": zsub})
        unshard.append(order)

    return in_maps, unshard, Ct, S


def _consts(Wq, Wk, We, Wv):
    Wq = np.asarray(Wq, np.float32); Wk = np.asarray(Wk, np.float32)
    We = np.asarray(We, np.float32); Wv = np.asarray(Wv, np.float32)
    wblk = np.zeros((128, 192), F16)
    wblk[0:64, 0:64] = We.T.astype(F16)
    wblk[64:128, 64:128] = Wk.T.astype(F16)
    wblk[64:128, 128:192] = Wv.T.astype(F16)
    wqt = Wq.T.astype(F16)
    ident = np.eye(128, dtype=F16)
    return {"wblk": wblk, "wqt": wqt, "ident": ident}


# ----------------------------------------------------------------------------
# device program
# ----------------------------------------------------------------------------

def _build_program(Ct, S, reps=1):
    nc = bacc.Bacc("TRN2", target_bir_lowering=False, debug=False,
                   num_devices=NCORES)
    f32 = mybir.dt.float32
    f16 = mybir.dt.float16

    stat_d = nc.dram_tensor("stat", [128, S], f16, kind="ExternalInput")
    xtq_d = nc.dram_tensor("xtq", [64, NPAD], f16, kind="ExternalInput")
    zsub_d = nc.dram_tensor("zsub", [128, NT], f32, kind="ExternalInput")
    wblk_d = nc.dram_tensor("wblk", [128, 192], f16, kind="ExternalInput")
    wqt_d = nc.dram_tensor("wqt", [64, 64], f16, kind="ExternalInput")
    ident_d = nc.dram_tensor("ident", [128, 128], f16, kind="ExternalInput")
    out_d = nc.dram_tensor("out", [128, NT * 64], f32, kind="ExternalOutput")

    Exp = mybir.ActivationFunctionType.Exp
    mult = mybir.AluOpType.mult
    addop = mybir.AluOpType.add
    off = np.r_[0, np.cumsum(Ct)].astype(int)

    def emit_pass(nc, constp, persp, streamp, workp, psump,
                  wblk, wqt, ident, zsub, xtq, q_sb, acc_sb):
        # ---- phase Q: Q[rank, :] = x[rank] @ Wq.T, node-rank-major ----
        for t in range(NT):
            qp = psump.tile([128, 68], f32, tag="acc_ps")
            nc.tensor.matmul(out=qp[:, 0:64],
                             lhsT=xtq[:, t * 128:(t + 1) * 128],
                             rhs=wqt[:], start=True, stop=True)
            nc.scalar.copy(q_sb[:, t * 64:(t + 1) * 64], qp[:, 0:64])

        # ---- main loop over node tiles ----
        for t in range(NT):
            C = int(Ct[t])
            av = acc_sb[:, t * 68:(t + 1) * 68]
            if C == 0:
                nc.vector.memset(av, 0.0)
                continue
            accp = psump.tile([128, 68], f32, tag="acc_ps")
            qt = q_sb[:, t * 64:(t + 1) * 64]
            nsub = (C + CSUB - 1) // CSUB
            for s in range(nsub):
                c0 = s * CSUB
                cs = min(CSUB, C - c0)
                col0 = (off[t] + c0) * 128
                st = streamp.tile([128, CSUB * 128], f16, tag="stat")
                nc.sync.dma_start(st[:, 0:cs * 128],
                                  stat_d[:, col0:col0 + cs * 128])
                # merged E|K matmul (wblk cols 0:128) -> [128, j, 0:64]=E,
                # [128, j, 64:128]=K; V separate (cols 128:192).
                n0 = min(4, cs)
                ekp0 = psump.tile([128, 4, 128], f32, tag="ek0")
                if cs > 4:
                    ekp1 = psump.tile([128, 4, 128], f32, tag="ek1")
                else:
                    ekp1 = None
                vp = psump.tile([128, CSUB * 64], f32, tag="V")
                for j in range(cs):
                    lh = st[:, j * 128:(j + 1) * 128]
                    ekp = ekp0 if j < 4 else ekp1
                    nc.tensor.matmul(out=ekp[:, j % 4, :], lhsT=lh,
                                     rhs=wblk[:, 0:128],
                                     start=True, stop=True)
                    nc.tensor.matmul(out=vp[:, j * 64:(j + 1) * 64],
                                     lhsT=lh, rhs=wblk[:, 128:192],
                                     start=True, stop=True)
                # drain E+K psum -> SBUF fp16 (scheduler balances ACT/DVE)
                ek_sb = workp.tile([128, CSUB, 128], f16, tag="ek_sb")
                nc.any.tensor_copy(ek_sb[:, 0:n0, :], ekp0[:, 0:n0, :])
                if cs > 4:
                    nc.any.tensor_copy(ek_sb[:, 4:cs, :],
                                       ekp1[:, 0:cs - 4, :])
                fd = cs * 64
                p2 = workp.tile([128, CSUB * 64], f16, tag="p2")
                nc.vector.tensor_tensor(
                    out=p2[:, 0:fd].rearrange("p (c d) -> p c d", d=64),
                    in0=ek_sb[:, 0:cs, 64:128],
                    in1=qt.unsqueeze(1).to_broadcast([128, cs, 64]),
                    op=mult)
                p3 = workp.tile([128, CSUB * 64], f16, tag="p3")
                p3eng = nc.gpsimd if P3_ON_POOL else nc.vector
                p3eng.tensor_tensor(
                    out=p3[:, 0:fd].rearrange("p (c d) -> p c d", d=64),
                    in0=p2[:, 0:fd].rearrange("p (c d) -> p c d", d=64),
                    in1=ek_sb[:, 0:cs, 0:64],
                    op=mult)
                sc = workp.tile([128, CSUB * 4], f32, tag="sc")
                if TREE_REDUCE:
                    p3v = p3[:, 0:fd].rearrange("p (c h d) -> p c h d",
                                                h=4, d=16)
                    r1 = workp.tile([128, CSUB * 32], f16, tag="r1")
                    r1v = r1[:, 0:cs * 32].rearrange("p (c h d) -> p c h d",
                                                     h=4, d=8)
                    nc.vector.tensor_tensor(out=r1v, in0=p3v[:, :, :, 0:8],
                                            in1=p3v[:, :, :, 8:16], op=addop)
                    r2 = workp.tile([128, CSUB * 16], f16, tag="r2")
                    r2v = r2[:, 0:cs * 16].rearrange("p (c h d) -> p c h d",
                                                     h=4, d=4)
                    nc.vector.tensor_tensor(out=r2v, in0=r1v[:, :, :, 0:4],
                                            in1=r1v[:, :, :, 4:8], op=addop)
                    r3 = workp.tile([128, CSUB * 8], f16, tag="r3")
                    r3v = r3[:, 0:cs * 8].rearrange("p (c h d) -> p c h d",
                                                    h=4, d=2)
                    nc.vector.tensor_tensor(out=r3v, in0=r2v[:, :, :, 0:2],
                                            in1=r2v[:, :, :, 2:4], op=addop)
                    nc.vector.tensor_tensor(
                        out=sc[:, 0:cs * 4].rearrange("p (c h) -> p c h", h=4)
                            .unsqueeze(3),
                        in0=r3v[:, :, :, 0:1],
                        in1=r3v[:, :, :, 1:2], op=addop)
                else:
                    nc.vector.tensor_reduce(
                        out=sc[:, 0:cs * 4].rearrange("p (c h) -> p c h", h=4),
                        in_=p3[:, 0:fd].rearrange("p (c h d) -> p c h d",
                                                  h=4, d=16),
                        axis=mybir.AxisListType.X, op=addop)
                scc = workp.tile([128, CSUB * 4], f32, tag="scc")
                nc.vector.tensor_scalar(
                    out=scc[:, 0:cs * 4], in0=sc[:, 0:cs * 4],
                    scalar1=-CLIP, scalar2=CLIP,
                    op0=mybir.AluOpType.max, op1=mybir.AluOpType.min)
                mb = workp.tile([128, CSUB, 68], f16, tag="mb")
                nc.scalar.activation(
                    out=mb[:, 0:cs, 64:68],
                    in_=scc[:, 0:cs * 4].rearrange("p (c h) -> p c h", h=4),
                    func=Exp, scale=float(1.0 / np.sqrt(DH)))
                nc.vector.tensor_tensor(
                    out=mb[:, 0:cs, 0:64].rearrange("p c (h d) -> p c h d",
                                                    h=4),
                    in0=vp[:, 0:fd].rearrange("p (c h d) -> p c h d",
                                              h=4, d=16),
                    in1=mb[:, 0:cs, 64:68].unsqueeze(3)
                        .to_broadcast([128, cs, 4, 16]),
                    op=mult)
                for j in range(cs):
                    nc.tensor.matmul(out=accp[:],
                                     lhsT=ident[:], rhs=mb[:, j, :],
                                     start=(c0 + j == 0),
                                     stop=(c0 + j == C - 1))
            nc.any.tensor_copy(av, accp[:])

        # ---- normalize: out = wV / (Z - zsub + 1e-6) ----
        accv = acc_sb[:].rearrange("p (t w) -> p t w", w=68)
        zc = persp.tile([128, NT * 4], f32, tag="zc")
        zcv = zc[:].rearrange("p (t h) -> p t h", h=4)
        nc.vector.tensor_tensor(
            out=zcv, in0=accv[:, :, 64:68],
            in1=zsub[:].unsqueeze(2).to_broadcast([128, NT, 4]),
            op=mybir.AluOpType.subtract)
        nc.vector.tensor_scalar_add(zc[:], zc[:], 1e-6)
        rz = persp.tile([128, NT * 4], f32, tag="rz")
        nc.vector.reciprocal(out=rz[:], in_=zc[:])
        nc.vector.tensor_tensor(
            out=accv[:, :, 0:64].rearrange("p t (h d) -> p t h d", h=4),
            in0=accv[:, :, 0:64].rearrange("p t (h d) -> p t h d", h=4),
            in1=rz[:].rearrange("p (t h) -> p t h", h=4).unsqueeze(3)
                  .to_broadcast([128, NT, 4, 16]),
            op=mult)
        nc.sync.dma_start(
            out_d.ap().rearrange("p (t d) -> p t d", d=64),
            accv[:, :, 0:64])

    with tile.TileContext(nc) as tc:
        with (
            tc.tile_pool(name="const", bufs=1) as constp,
            tc.tile_pool(name="pers", bufs=1) as persp,
            tc.tile_pool(name="stream", bufs=3) as streamp,
            tc.tile_pool(name="work", bufs=2) as workp,
            tc.tile_pool(name="psum", bufs=2, space="PSUM") as psump,
        ):
            wblk = constp.tile([128, 192], f16, tag="wblk")
            nc.sync.dma_start(wblk[:], wblk_d[:])
            wqt = constp.tile([64, 64], f16, tag="wqt")
            nc.sync.dma_start(wqt[:], wqt_d[:])
            ident = constp.tile([128, 128], f16, tag="ident")
            nc.sync.dma_start(ident[:], ident_d[:])
            zsub = constp.tile([128, NT], f32, tag="zsub")
            nc.sync.dma_start(zsub[:], zsub_d[:])

            xtq = persp.tile([64, NPAD], f16, tag="xtq")
            nc.sync.dma_start(xtq[:], xtq_d[:])
            q_sb = persp.tile([128, NT * 64], f16, tag="q")
            acc_sb = persp.tile([128, NT * 68], f32, tag="acc")

            for _rep in range(reps):
                emit_pass(nc, constp, persp, streamp, workp, psump,
                          wblk, wqt, ident, zsub, xtq, q_sb, acc_sb)

    nc.compile()
    return nc


# ----------------------------------------------------------------------------
# public entry point
# ----------------------------------------------------------------------------

_CACHE = {}


def prepare(x, edge_attr, Wq, Wk, We, Wv, edge_index, reps=1):
    """Host prep + (cached) program build.

    Returns (nc, in_maps, unshard, Ct, S)."""
    in_maps, unshard, Ct, S = _shard(x, edge_attr, edge_index)
    consts = _consts(Wq, Wk, We, Wv)
    for m in in_maps:
        m.update(consts)
    key = (tuple(Ct), S, reps)
    if key not in _CACHE:
        _CACHE[key] = _build_program(Ct, S, reps=reps)
    return _CACHE[key], in_maps, unshard, Ct, S


def unshard_output(results, unshard):
    h = np.empty((N_NODES, DIM), np.float32)
    for c in range(NCORES):
        o = results[c]["out"].reshape(128, NT, 64)
        r = np.arange(NPC)
        rows = o[r % 128, r // 128]          # [NPC, 64] by rank
        h[c * NPC + unshard[c]] = rows
    return h


def kernel(x, edge_attr, Wq, Wk, We, Wv, edge_index):
    nc, in_maps, unshard, _, _ = prepare(x, edge_attr, Wq, Wk, We, Wv,
                                         edge_index)
    res = run_bass_kernel_spmd(nc, in_maps, core_ids=list(range(NCORES)))
    return unshard_output(res.results, unshard)
